# revision 5
# baseline (speedup 1.0000x reference)
"""Trainium2 Bass kernel for nn_CNN_PHMM_VAE loss (profile-HMM forward + VAE KLD).

Strategy: pure data parallel over 8 NeuronCores (64 examples per core).
The PHMM forward runs in probability space with periodic rescaling. Per
sequence step the DVE critical path is 6 instructions (no cross-engine
dependencies inside the loop — semaphore overhead dominates any offload):
  Z     = W3 o [mu_a,mu_b,y]  one fused 3x65 multiply: beta (scan input),
                              r1=(G1-G2)*mu, r2=G2*y
  delta = scan(V, Z[0:65])    delete-state column (affine scan, fp32 carry)
  t     = y + delta           match input mix
  mu'   = ee_l o t            emission multiply, dual-written to mu_a/mu_b
                              (ee is a host-gathered table, DMA'd as bf16)
  ya    = r1 + r2             insert-state update
  y'    = mu' + ya
All per-example constant tables AND the O(B*L*K) emission gather table
ee[b,l,k] = A1[b,k+1]*exp(emission[b,k,s[b,l]]) are precomputed host-side;
the sequential dynamic program runs on device.

Layout per core: 64 examples on partitions; profile positions k on the
free dimension. State X = [mu(65)|pad|mu(65)|pad|y(65)|pads], bf16; the
duplicated mu makes the fused Z multiply's source access pattern affine.
"""
import numpy as np

B, L, K, E = 512, 256, 64, 16
K1 = K + 1
N_CORES = 8
BPC = B // N_CORES  # 64 examples per core
R = 16              # rescale interval (steps)
LOGACC0 = -60.0     # initial global log-scale
NEG = -100.0
M2M, M2I, M2D, I2M, I2I, D2M, D2D = 0, 1, 2, 3, 4, 5, 6

# --- small f32 table layout (free-dim offsets, per partition/example) -------
OFF_X0 = 0             # 200: initial [mu | pad | mu | pad | y | pads]
OFF_W3 = 200           # 200: rows of 66: [U1 | G1-G2 | G2]
OFF_V = 400            # 65
OFF_A1C0 = 466         # 1
OFF_A3C0 = 467         # 1
OFF_SIG0 = 468         # 1
OFF_MUS = 470          # 16
OFF_LV = 486           # 16
TBL_W = 502

XW = 200   # state: mu_a 0..64, mu_b 66..130, y 132..196
ZW = 200   # Z: seed@0, beta 1..65, r1 67..131, r2 133..197

INSTRUMENT = True  # append scratch op-cost probes after the output DMA

_CACHED = {}


def _host_tables(batch_input, transition_probs, emission_probs, mus, logvars):
    """Per-example constant tables, computed in float64, stored float32/bf16."""
    import ml_dtypes

    a = np.asarray(transition_probs, np.float64)
    Earr = np.exp(np.asarray(emission_probs, np.float64))  # (B, K, 4)
    s = np.asarray(batch_input)
    A1 = np.exp(a[:, :, M2M])
    A2 = np.exp(a[:, :, I2M])
    A3 = np.exp(a[:, :, D2M])
    B1 = 0.25 * np.exp(a[:, :, M2I])
    B2 = 0.25 * np.exp(a[:, :, I2I])
    C1 = np.exp(a[:, :, M2D])
    C2 = np.exp(a[:, :, D2D])

    U = np.zeros((B, K1)); V = np.zeros((B, K1))
    U[:, 1:] = A3[:, 1:] * C1[:, :-1] / A1[:, :-1]
    V[:, 1:] = A3[:, 1:] * C2[:, :-1] / A3[:, :-1]
    G1 = A2 * B1 / A1
    G2 = B2

    # ee[b, l, j] = A1[b, j+1] * exp(emission[b, j, s[b, l]]), j = 0..K-1
    Etil = A1[:, 1:, None] * Earr  # (B, K, 4)
    ee = Etil[np.arange(B)[:, None, None], np.arange(K)[None, None, :],
              s[:, :, None]]  # (B, L, K)
    ee_bf = np.asarray(ee, ml_dtypes.bfloat16).reshape(B, L * K)

    sig0 = np.exp(NEG - LOGACC0)          # e^-40, scaled NEG seed
    e0 = np.exp(-LOGACC0)                 # e^60, scaled "1.0"
    mu0 = np.empty((B, K1)); iot0 = np.empty((B, K1))
    mu0[:, 0] = A1[:, 0] * e0
    mu0[:, 1:] = A1[:, 1:] * sig0
    iot0[:, :] = A2 * sig0

    tbl = np.zeros((B, TBL_W), np.float32)
    tbl[:, OFF_X0:OFF_X0 + K1] = mu0
    tbl[:, OFF_X0 + 66:OFF_X0 + 66 + K1] = mu0
    tbl[:, OFF_X0 + 132:OFF_X0 + 132 + K1] = mu0 + iot0
    tbl[:, OFF_W3:OFF_W3 + K] = U[:, 1:]          # U1[i] = U[i+1]; slot 64 = 0
    tbl[:, OFF_W3 + 66:OFF_W3 + 66 + K1] = G1 - G2
    tbl[:, OFF_W3 + 132:OFF_W3 + 132 + K1] = G2
    tbl[:, OFF_V:OFF_V + K1] = V
    tbl[:, OFF_A1C0] = A1[:, 0]
    tbl[:, OFF_A3C0] = A3[:, 0]
    tbl[:, OFF_SIG0] = sig0
    tbl[:, OFF_MUS:OFF_MUS + E] = np.asarray(mus, np.float32)
    tbl[:, OFF_LV:OFF_LV + E] = np.asarray(logvars, np.float32)
    return tbl, ee_bf


def _build_bass():
    import concourse.bass as bass
    import concourse.tile as tile
    from concourse import bacc, mybir
    from concourse.ap import AP
    from contextlib import ExitStack

    f32 = mybir.dt.float32
    bf = mybir.dt.bfloat16
    mult = mybir.AluOpType.mult
    add = mybir.AluOpType.add
    mx_op = mybir.AluOpType.max
    AF = mybir.ActivationFunctionType

    nc = bacc.Bacc("TRN2", target_bir_lowering=False, debug=False,
                   num_devices=N_CORES)
    tbl_d = nc.dram_tensor("tbl", [BPC, TBL_W], f32, kind="ExternalInput").ap()
    ee_d = nc.dram_tensor("ee", [BPC, L * K], bf, kind="ExternalInput").ap()
    out_d = nc.dram_tensor("loss", [BPC, 1], f32, kind="ExternalOutput").ap()

    def strided(ap, dims):
        """Custom multi-dim free AP on the same tensor/offset."""
        return AP(ap.tensor, ap.offset, [list(ap.ap[0])] + dims)

    with tile.TileContext(nc) as tc, ExitStack() as ctx:
        ctx.enter_context(nc.allow_low_precision(
            reason="bf16 DP state validated to ~2e-5 relative on the loss"))
        pool = ctx.enter_context(tc.tile_pool(name="p", bufs=1))

        TBL = pool.tile([BPC, TBL_W], f32, tag="TBL", name="TBL")
        EEt = pool.tile([BPC, L * K], bf, tag="EE", name="EE")
        nc.sync.dma_start(TBL[:, :], tbl_d[:, :])
        NCH = 4
        CW = L * K // NCH
        for c in range(NCH):
            nc.sync.dma_start(EEt[:, c * CW:(c + 1) * CW],
                              ee_d[:, c * CW:(c + 1) * CW])

        def tb(off, n):
            return TBL[:, off:off + n]

        v = nc.vector

        # KLD (one-time, before the DP loop)
        ev = pool.tile([BPC, E], f32, tag="ev", name="ev")
        sq = pool.tile([BPC, E], f32, tag="sq", name="sq")
        w1 = pool.tile([BPC, E], f32, tag="w1", name="w1")
        w2 = pool.tile([BPC, E], f32, tag="w2", name="w2")
        red = pool.tile([BPC, 1], f32, tag="red", name="red")
        kld = pool.tile([BPC, 1], f32, tag="kld", name="kld")
        nc.scalar.activation(ev[:, :], tb(OFF_LV, E), AF.Exp)
        nc.scalar.activation(sq[:, :], tb(OFF_MUS, E), AF.Square)
        v.tensor_sub(w1[:, :], tb(OFF_LV, E), sq[:, :])
        v.tensor_sub(w2[:, :], w1[:, :], ev[:, :])
        v.tensor_reduce(red[:, :], w2[:, :], mybir.AxisListType.X, add)
        v.tensor_scalar(kld[:, :], red[:, :], -0.5, -float(E) / 2.0, mult, add)

        # state + work tiles (bf16)
        x_pp = [pool.tile([BPC, XW], bf, tag="x_a", name="x_a"),
                pool.tile([BPC, XW], bf, tag="x_b", name="x_b")]
        ZT = pool.tile([BPC, 2 * ZW], bf, tag="ZT", name="ZT")  # ping-pong
        W3b = pool.tile([BPC, ZW], bf, tag="W3b", name="W3b")
        Vb = pool.tile([BPC, K1], bf, tag="Vb", name="Vb")
        delta = pool.tile([BPC, K1], bf, tag="delta", name="delta")
        t = pool.tile([BPC, K], bf, tag="t", name="t")
        ya = pool.tile([BPC, K1], bf, tag="ya", name="ya")
        sig = pool.tile([BPC, 1], f32, tag="sig", name="sig")
        rmxb = pool.tile([BPC, 1], bf, tag="rmxb", name="rmxb")
        mxt = pool.tile([BPC, 1], f32, tag="mxt", name="mxt")
        NRS = L // R - 1
        rhist = pool.tile([BPC, NRS], f32, tag="rhist", name="rhist")

        # init: cast tables to bf16, seed state
        v.memset(x_pp[0][:, :], 0.0)
        v.memset(x_pp[1][:, :], 0.0)
        v.memset(ZT[:, :], 0.0)
        v.tensor_copy(W3b[:, :], tb(OFF_W3, ZW))
        v.tensor_copy(Vb[:, :], tb(OFF_V, K1))
        v.tensor_copy(x_pp[0][:, :], tb(OFF_X0, XW))
        v.tensor_copy(sig[:, :], tb(OFF_SIG0, 1))
        # Z seeds (both halves) = A3[0]*sig; stale-buffer mu seeds = A1[0]*sig
        zseed = strided(ZT[:, 0:1], [[ZW, 2], [1, 1]])
        bc2 = lambda ap: ap.unsqueeze(1).broadcast_to((BPC, 2, 1))
        v.tensor_mul(zseed, bc2(tb(OFF_A3C0, 1)), bc2(sig[:, :]))
        v.tensor_mul(strided(x_pp[1][:, 0:1], [[66, 2], [1, 1]]),
                     bc2(tb(OFF_A1C0, 1)), bc2(sig[:, :]))

        def dp_step(l):
            X, Xn = x_pp[l % 2], x_pp[(l + 1) % 2]
            Z = ZT[:, (l % 2) * ZW:(l % 2) * ZW + ZW]
            # Z = W3 o [mu_a, mu_b, y]: iter (3,65), outer stride 66
            v.tensor_mul(strided(Z[:, 1:2], [[66, 3], [1, K1]]),
                         strided(W3b[:, 0:1], [[66, 3], [1, K1]]),
                         strided(X[:, 0:1], [[66, 3], [1, K1]]))
            v.tensor_tensor_scan(delta[:, :], Vb[:, :], Z[:, 0:K1], 0.0, mult, add)
            v.tensor_add(t[:, :], X[:, 132:132 + K], delta[:, 0:K])
            ee_l = EEt[:, l * K:(l + 1) * K]
            # mu' dual-written into mu_a and mu_b slots
            v.tensor_mul(strided(Xn[:, 1:2], [[66, 2], [1, K]]),
                         ee_l.unsqueeze(1).broadcast_to((BPC, 2, K)),
                         t[:, :].unsqueeze(1).broadcast_to((BPC, 2, K)))
            v.tensor_add(ya[:, :], Z[:, 67:67 + K1], Z[:, 133:133 + K1])
            v.tensor_add(Xn[:, 132:132 + K1], Xn[:, 0:K1], ya[:, :])

        def rescale(i, l):
            cur = (l + 1) % 2
            Xn = x_pp[cur]
            x_stale = x_pp[1 - cur]
            rmx = rhist[:, i:i + 1]
            # y >= mu and y >= iota elementwise, so max(y) is the state max
            v.tensor_reduce(mxt[:, :], Xn[:, 132:132 + K1], mybir.AxisListType.X,
                            mx_op)
            v.reciprocal(rmxb[:, :], mxt[:, :])   # quantize factor to bf16
            v.tensor_copy(rmx, rmxb[:, :])        # record exact applied factor
            v.tensor_scalar_mul(Xn[:, :], Xn[:, :], rmx)
            v.tensor_scalar_mul(sig[:, :], sig[:, :], rmx)
            v.tensor_mul(zseed, bc2(tb(OFF_A3C0, 1)), bc2(sig[:, :]))
            v.tensor_mul(strided(x_stale[:, 0:1], [[66, 2], [1, 1]]),
                         bc2(tb(OFF_A1C0, 1)), bc2(sig[:, :]))

        for l in range(L):
            dp_step(l)
            if l == 0:
                # column 0 of the l=0 buffer carried the "M0[0]=1" seed;
                # columns >= 1 reseed with sigma (the NEG=-100 re-injection)
                v.tensor_mul(strided(x_pp[0][:, 0:1], [[66, 2], [1, 1]]),
                             bc2(tb(OFF_A1C0, 1)), bc2(sig[:, :]))
            if (l + 1) % R == 0 and (l + 1) < L:
                rescale((l + 1) // R - 1, l)

        # final column: buffer L % 2 = 0
        Xf = x_pp[L % 2]
        Zf = ZT[:, 0:ZW]
        tf = pool.tile([BPC, K1], f32, tag="tf", name="tf")
        lnp = pool.tile([BPC, 1], f32, tag="lnp", name="lnp")
        lnr = pool.tile([BPC, NRS], f32, tag="lnr", name="lnr")
        sumlr = pool.tile([BPC, 1], f32, tag="sumlr", name="sumlr")
        lacc = pool.tile([BPC, 1], f32, tag="lacc", name="lacc")
        nv = pool.tile([BPC, 1], f32, tag="nv", name="nv")
        v.tensor_mul(Zf[:, 1:1 + K1], W3b[:, 0:K1], Xf[:, 0:K1])
        v.tensor_tensor_scan(delta[:, :], Vb[:, :], Zf[:, 0:K1], 0.0, mult, add)
        v.tensor_add(tf[:, :], Xf[:, 132:132 + K1], delta[:, :])
        nc.scalar.activation(lnp[:, :], tf[:, K:K1], AF.Ln)
        # lacc = LOGACC0 - sum_i ln(rmx_i)
        nc.scalar.activation(lnr[:, :], rhist[:, :], AF.Ln)
        v.tensor_reduce(sumlr[:, :], lnr[:, :], mybir.AxisListType.X, add)
        v.tensor_scalar(lacc[:, :], sumlr[:, :], -1.0, LOGACC0, mult, add)
        v.tensor_add(nv[:, :], lnp[:, :], lacc[:, :])  # = -nll
        loss_t = pool.tile([BPC, 1], f32, tag="loss_t", name="loss_t")
        v.tensor_sub(loss_t[:, :], kld[:, :], nv[:, :])  # kld + nll
        nc.sync.dma_start(out_d[:, :], loss_t[:, :])

        if INSTRUMENT:
            # Scratch op-cost probes: 4 reps each, serialized by reading the
            # previous probe's output. Identified in the trace by position
            # (the final Vector slices). Adds ~8us to this run only.
            SC = pool.tile([BPC, 600], bf, tag="SC", name="SC")
            SCf = pool.tile([BPC, 4], f32, tag="SCf", name="SCf")
            v.memset(SC[:, :], 1.0)
            v.tensor_copy(SCf[:, :], SC[:, 0:4])
            A0, B0 = SC[:, 0:300], SC[:, 300:600]

            def probe(fn, reps=4):
                for _ in range(reps):
                    fn()

            # p1: mul w64 dest offset 0
            probe(lambda: v.tensor_mul(SC[:, 300:364], SC[:, 0:64], SC[:, 64:128]))
            # p2: mul w64 dest offset 1 (odd)
            probe(lambda: v.tensor_mul(SC[:, 1:65], SC[:, 300:364], SC[:, 64:128]))
            # p3: mul w64 dest offset 2
            probe(lambda: v.tensor_mul(SC[:, 2:66], SC[:, 300:364], SC[:, 64:128]))
            # p4: 3-row mul [66,3],[1,65] dest@301 (195e)
            probe(lambda: v.tensor_mul(
                strided(SC[:, 301:302], [[66, 3], [1, 65]]),
                strided(SC[:, 0:1], [[66, 3], [1, 65]]),
                strided(SC[:, 2:3], [[66, 3], [1, 65]])))
            # p5: 4-row mul [[132,2],[66,2],[1,65]] dest@1 (260e)
            probe(lambda: v.tensor_mul(
                strided(SC[:, 1:2], [[132, 2], [66, 2], [1, 65]]),
                strided(SC[:, 300:301], [[132, 2], [66, 2], [1, 65]]),
                strided(SC[:, 302:303], [[66, 2], [0, 2], [1, 65]])))
            # p6: 2-row mul [[66,2],[1,65]] stride-0 src (130e)
            probe(lambda: v.tensor_mul(
                strided(SC[:, 300:301], [[66, 2], [1, 65]]),
                strided(SC[:, 1:2], [[66, 2], [1, 65]]),
                strided(SC[:, 3:4], [[0, 2], [1, 65]])))
            # p7: dual-write mul [[66,2],[1,64]] dest@1, broadcast srcs (128e)
            probe(lambda: v.tensor_mul(
                strided(SC[:, 1:2], [[66, 2], [1, 64]]),
                SC[:, 300:364].unsqueeze(1).broadcast_to((BPC, 2, 64)),
                SC[:, 400:464].unsqueeze(1).broadcast_to((BPC, 2, 64))))
            # p8: add w65 all-even
            probe(lambda: v.tensor_add(SC[:, 300:365], SC[:, 0:65], SC[:, 66:131]))
            # p9: scan w65
            probe(lambda: v.tensor_tensor_scan(SC[:, 0:65], SC[:, 300:365],
                                               SC[:, 366:431], 0.0, mult, add))
            # p10: stt w65 with f32 AP scalar
            probe(lambda: v.scalar_tensor_tensor(SC[:, 300:365], SC[:, 0:65],
                                                 SCf[:, 0:1], SC[:, 66:131],
                                                 mult, add))
            # p11: tensor_scalar_mul w200 f32 AP scalar
            probe(lambda: v.tensor_scalar_mul(SC[:, 0:200], SC[:, 0:200],
                                              SCf[:, 1:2]))
            # p12: add w64 srcs 66/0 dest 0 (t-add replica)
            probe(lambda: v.tensor_add(SC[:, 300:364], SC[:, 66:130],
                                       SC[:, 0:64]))

    nc.compile()
    return nc


def _get_nc():
    if "nc" not in _CACHED:
        _CACHED["nc"] = _build_bass()
    return _CACHED["nc"]


def kernel(batch_input, transition_probs, emission_probs, mus, logvars):
    from concourse.bass_utils import run_bass_kernel_spmd

    tbl, ee = _host_tables(batch_input, transition_probs, emission_probs,
                           mus, logvars)
    nc = _get_nc()
    in_maps = [{"tbl": tbl[c * BPC:(c + 1) * BPC],
                "ee": ee[c * BPC:(c + 1) * BPC]} for c in range(N_CORES)]
    res = run_bass_kernel_spmd(nc, in_maps, list(range(N_CORES)))
    losses = np.concatenate([np.asarray(r["loss"])[:, 0] for r in res.results])
    return np.float32(np.mean(losses.astype(np.float64)))
